# revision 5
# baseline (speedup 1.0000x reference)
"""Trainium2 kernel for BinaryLinear: out = x @ sign(clip(weight,-1,1)).T + bias.

Full shapes: x [8192, 4096] f32, weight [4096, 4096] f32, bias [4096] f32,
out [8192, 4096] f32.

Strategy (8 NeuronCores, no collectives needed):
  - 4 x 2 grid: shard tokens into 4 blocks of 2048, out_features into 2
    blocks of 2048. Each core computes a disjoint [2048, 2048] output tile.
  - Binarized weights are exactly +-1, representable in bf16. x is split
    hi/lo into two bf16 operands (x ~= hi + lo, ~17 mantissa bits), so two
    bf16 matmul passes accumulated in fp32 PSUM reproduce fp32 accuracy.
  - Host packs x transposed+tiled so every DMA is contiguous at line rate:
    the contraction dim (in_features) must sit on SBUF partitions for the
    PE, and packing on the host avoids burning PE/DMA time on transposes.
  - Per core: resident binarized-transposed weight slice in SBUF (16 MiB),
    stream 128-token blocks of xT, accumulate over K in 4 PSUM banks,
    add bias on DVE while copying PSUM->SBUF, DMA out.
"""

import sys

if "/opt/trn_rl_repo" not in sys.path:
    sys.path.insert(0, "/opt/trn_rl_repo")

import ml_dtypes
import numpy as np

N_TOK, D_IN, D_OUT = 8192, 4096, 4096
TOK_SHARDS, OUT_SHARDS = 4, 2
N_CORES = TOK_SHARDS * OUT_SHARDS
TOK_C = N_TOK // TOK_SHARDS  # 2048 tokens per core
OUT_C = D_OUT // OUT_SHARDS  # 2048 out_features per core
MB = TOK_C // 128  # 16 token blocks
KB = D_IN // 128  # 32 contraction blocks
NF = 512  # matmul moving free dim (one fp32 PSUM bank)
NB = OUT_C // NF  # 4 PSUM banks per token block

SPLIT = True  # hi/lo bf16 split (2 matmul passes) vs single bf16 pass

_cached_nc = None


def build_nc():
    import concourse.bacc as bacc
    import concourse.mybir as mybir
    import concourse.tile as tile

    nc = bacc.Bacc()
    dt = mybir.dt
    xh_d = nc.dram_tensor("xh", [MB, 128, D_IN], dt.bfloat16, kind="ExternalInput")
    if SPLIT:
        xl_d = nc.dram_tensor("xl", [MB, 128, D_IN], dt.bfloat16, kind="ExternalInput")
    wt_d = nc.dram_tensor("wt", [KB, 128, OUT_C], dt.bfloat16, kind="ExternalInput")
    br_d = nc.dram_tensor("br", [128, OUT_C], dt.float32, kind="ExternalInput")
    out_d = nc.dram_tensor("out", [TOK_C, OUT_C], dt.float32, kind="ExternalOutput")

    with tile.TileContext(nc) as tc:
        with (
            tc.tile_pool(name="wts", bufs=1) as wpool,
            tc.tile_pool(name="bias", bufs=1) as bpool,
            tc.tile_pool(name="xin", bufs=2) as xpool,
            tc.tile_pool(name="outp", bufs=2) as opool,
            tc.tile_pool(name="psum", bufs=8, space="PSUM") as ppool,
        ):
            wts = []
            for kb in range(KB):
                w = wpool.tile(
                    [128, OUT_C], dt.bfloat16, name=f"wt{kb}", tag=f"wt{kb}"
                )
                nc.sync.dma_start(w[:], wt_d[kb])
                wts.append(w)
            bias_s = bpool.tile([128, OUT_C], dt.float32, name="bias_s")
            nc.sync.dma_start(bias_s[:], br_d[:])

            for m in range(MB):
                xh_m = xpool.tile([128, D_IN], dt.bfloat16, name=f"xh_{m}", tag="xh")
                nc.sync.dma_start(xh_m[:], xh_d[m])
                passes = [xh_m]
                if SPLIT:
                    xl_m = xpool.tile(
                        [128, D_IN], dt.bfloat16, name=f"xl_{m}", tag="xl"
                    )
                    nc.sync.dma_start(xl_m[:], xl_d[m])
                    passes.append(xl_m)

                ps = [
                    ppool.tile([128, NF], dt.float32, name=f"ps_{m}_{n}", tag="ps")
                    for n in range(NB)
                ]
                n_half = len(passes)
                for kb in range(KB):
                    for hi, xm in enumerate(passes):
                        lhs = xm[:, kb * 128 : (kb + 1) * 128]
                        for n in range(NB):
                            nc.tensor.matmul(
                                ps[n][:],
                                lhs,
                                wts[kb][:, n * NF : (n + 1) * NF],
                                start=(kb == 0 and hi == 0),
                                stop=(kb == KB - 1 and hi == n_half - 1),
                            )

                out_t = opool.tile([128, OUT_C], dt.float32, name=f"o_{m}", tag="out")
                for n in range(NB):
                    nc.vector.tensor_tensor(
                        out_t[:, n * NF : (n + 1) * NF],
                        ps[n][:],
                        bias_s[:, n * NF : (n + 1) * NF],
                        mybir.AluOpType.add,
                    )
                nc.sync.dma_start(out_d[m * 128 : (m + 1) * 128, :], out_t[:])

    nc.compile()
    return nc


def _pack_x(a):
    """[TOK_C, D_IN] -> [MB, 128, D_IN] with layout [m, p, (kb t)]:
    packed[m, p, kb*128 + t] = a[m*128 + t, kb*128 + p]."""
    return np.ascontiguousarray(
        a.reshape(MB, 128, KB, 128).transpose(0, 3, 2, 1)
    ).reshape(MB, 128, D_IN)


def prepare_in_maps(x, weight, bias):
    x = np.asarray(x, dtype=np.float32)
    weight = np.asarray(weight, dtype=np.float32)
    bias = np.asarray(bias, dtype=np.float32)

    bw = np.where(weight >= 0, np.float32(1.0), np.float32(-1.0))

    # per-out-shard packed weights and bias (shared across token shards)
    wt_packs, bias_packs = [], []
    for oi in range(OUT_SHARDS):
        w_sh = bw[oi * OUT_C : (oi + 1) * OUT_C]  # [OUT_C, D_IN]
        wt = np.ascontiguousarray(w_sh.T).astype(ml_dtypes.bfloat16)
        wt_packs.append(wt.reshape(KB, 128, OUT_C))
        bias_packs.append(
            np.ascontiguousarray(
                np.broadcast_to(bias[oi * OUT_C : (oi + 1) * OUT_C], (128, OUT_C))
            )
        )

    # per-token-shard packed x hi/lo (shared across out shards)
    xh_packs, xl_packs = [], []
    for ti in range(TOK_SHARDS):
        x_sh = x[ti * TOK_C : (ti + 1) * TOK_C]
        xh = x_sh.astype(ml_dtypes.bfloat16)
        xh_packs.append(_pack_x(xh))
        if SPLIT:
            xl = (x_sh - xh.astype(np.float32)).astype(ml_dtypes.bfloat16)
            xl_packs.append(_pack_x(xl))

    in_maps = []
    for c in range(N_CORES):
        ti, oi = divmod(c, OUT_SHARDS)
        m = {"xh": xh_packs[ti], "wt": wt_packs[oi], "br": bias_packs[oi]}
        if SPLIT:
            m["xl"] = xl_packs[ti]
        in_maps.append(m)
    return in_maps


def run(in_maps, trace=False, **kwargs):
    global _cached_nc
    from concourse.bass_utils import run_bass_kernel_spmd

    if _cached_nc is None:
        _cached_nc = build_nc()
    return run_bass_kernel_spmd(
        _cached_nc, in_maps, list(range(N_CORES)), trace=trace, **kwargs
    )


def gather(results):
    out = np.empty((N_TOK, D_OUT), dtype=np.float32)
    for c in range(N_CORES):
        ti, oi = divmod(c, OUT_SHARDS)
        out[ti * TOK_C : (ti + 1) * TOK_C, oi * OUT_C : (oi + 1) * OUT_C] = results[c][
            "out"
        ]
    return out


def kernel(x, weight, bias):
    res = run(prepare_in_maps(x, weight, bias), trace=False)
    return gather(res.results)


# revision 6
# speedup vs baseline: 1.7665x; 1.7665x over previous
"""Trainium2 kernel for BinaryLinear: out = x @ sign(clip(weight,-1,1)).T + bias.

Full shapes: x [8192, 4096] f32, weight [4096, 4096] f32, bias [4096] f32,
out [8192, 4096] f32.

Strategy (8 NeuronCores, no collectives needed):
  - Grid-shard tokens x out_features across the 8 cores; each core computes
    a disjoint output tile, host slices inputs / stitches outputs.
  - Binarized weights are exactly +-1 (bf16/f32r-exact). The matmul runs
    on the PE at 1 cycle/row using float32r operands (f32 bits, reduced-
    precision multiplier, ~2^-13 per-term error -> ~1e-4 rel overall).
  - Host packs x transposed+tiled so the contraction dim (in_features)
    lands on SBUF partitions with every DMA contiguous at line rate.
  - Per core: resident binarized-transposed weight slice in SBUF,
    stream 128-token blocks of xT, accumulate over K=4096 in PSUM,
    add bias on DVE while copying PSUM->SBUF, DMA out.

MODE:
  "f32r"  : single pass, f32r x f32r, 2x4 grid (tok x outf). ~1e-4 rel.
  "bf16x2": x split hi/lo into two bf16 passes, 4x2 grid. ~2e-6 rel,
            about 1.8x slower.
"""

import sys

if "/opt/trn_rl_repo" not in sys.path:
    sys.path.insert(0, "/opt/trn_rl_repo")

import ml_dtypes
import numpy as np

MODE = "f32r"

N_TOK, D_IN, D_OUT = 8192, 4096, 4096
if MODE == "f32r":
    TOK_SHARDS, OUT_SHARDS = 2, 4
else:
    TOK_SHARDS, OUT_SHARDS = 4, 2
N_CORES = TOK_SHARDS * OUT_SHARDS
TOK_C = N_TOK // TOK_SHARDS
OUT_C = D_OUT // OUT_SHARDS
MB = TOK_C // 128  # token blocks per core
KB = D_IN // 128  # contraction blocks
NF = 512  # matmul moving free dim (one fp32 PSUM bank)
NB = OUT_C // NF  # PSUM banks per token block

_cached_nc = None


def build_nc():
    import concourse.bacc as bacc
    import concourse.mybir as mybir
    import concourse.tile as tile

    dt = mybir.dt
    split = MODE == "bf16x2"
    mdt = dt.bfloat16 if split else dt.float32r

    nc = bacc.Bacc()
    xh_d = nc.dram_tensor("xh", [MB, 128, D_IN], mdt, kind="ExternalInput")
    if split:
        xl_d = nc.dram_tensor("xl", [MB, 128, D_IN], mdt, kind="ExternalInput")
    wt_d = nc.dram_tensor("wt", [KB, 128, OUT_C], mdt, kind="ExternalInput")
    br_d = nc.dram_tensor("br", [128, OUT_C], dt.float32, kind="ExternalInput")
    out_d = nc.dram_tensor("out", [TOK_C, OUT_C], dt.float32, kind="ExternalOutput")

    with tile.TileContext(nc) as tc:
        with (
            tc.tile_pool(name="wts", bufs=1) as wpool,
            tc.tile_pool(name="bias", bufs=1) as bpool,
            tc.tile_pool(name="xin", bufs=2) as xpool,
            tc.tile_pool(name="outp", bufs=2) as opool,
            tc.tile_pool(name="psum", bufs=8, space="PSUM") as ppool,
        ):
            wts = []
            for kb in range(KB):
                w = wpool.tile([128, OUT_C], mdt, name=f"wt{kb}", tag=f"wt{kb}")
                nc.sync.dma_start(w[:], wt_d[kb])
                wts.append(w)
            bias_s = bpool.tile([128, OUT_C], dt.float32, name="bias_s")
            nc.sync.dma_start(bias_s[:], br_d[:])

            for m in range(MB):
                xh_m = xpool.tile([128, D_IN], mdt, name=f"xh_{m}", tag="xh")
                nc.sync.dma_start(xh_m[:], xh_d[m])
                passes = [xh_m]
                if split:
                    xl_m = xpool.tile([128, D_IN], mdt, name=f"xl_{m}", tag="xl")
                    nc.sync.dma_start(xl_m[:], xl_d[m])
                    passes.append(xl_m)

                ps = [
                    ppool.tile([128, NF], dt.float32, name=f"ps_{m}_{n}", tag="ps")
                    for n in range(NB)
                ]
                n_half = len(passes)
                for kb in range(KB):
                    for hi, xm in enumerate(passes):
                        lhs = xm[:, kb * 128 : (kb + 1) * 128]
                        for n in range(NB):
                            nc.tensor.matmul(
                                ps[n][:],
                                lhs,
                                wts[kb][:, n * NF : (n + 1) * NF],
                                start=(kb == 0 and hi == 0),
                                stop=(kb == KB - 1 and hi == n_half - 1),
                            )

                out_t = opool.tile([128, OUT_C], dt.float32, name=f"o_{m}", tag="out")
                for n in range(NB):
                    nc.vector.tensor_tensor(
                        out_t[:, n * NF : (n + 1) * NF],
                        ps[n][:],
                        bias_s[:, n * NF : (n + 1) * NF],
                        mybir.AluOpType.add,
                    )
                nc.sync.dma_start(out_d[m * 128 : (m + 1) * 128, :], out_t[:])

    nc.compile()
    return nc


def _pack_x(a):
    """[TOK_C, D_IN] -> [MB, 128, D_IN] with layout [m, p, (kb t)]:
    packed[m, p, kb*128 + t] = a[m*128 + t, kb*128 + p]."""
    return np.ascontiguousarray(
        a.reshape(MB, 128, KB, 128).transpose(0, 3, 2, 1)
    ).reshape(MB, 128, D_IN)


def prepare_in_maps(x, weight, bias):
    x = np.asarray(x, dtype=np.float32)
    weight = np.asarray(weight, dtype=np.float32)
    bias = np.asarray(bias, dtype=np.float32)
    split = MODE == "bf16x2"
    npdt = ml_dtypes.bfloat16 if split else np.float32

    bw = np.where(weight >= 0, np.float32(1.0), np.float32(-1.0))

    wt_packs, bias_packs = [], []
    for oi in range(OUT_SHARDS):
        w_sh = bw[oi * OUT_C : (oi + 1) * OUT_C]  # [OUT_C, D_IN]
        wt = np.ascontiguousarray(w_sh.T).astype(npdt)
        wt_packs.append(wt.reshape(KB, 128, OUT_C))
        bias_packs.append(
            np.ascontiguousarray(
                np.broadcast_to(bias[oi * OUT_C : (oi + 1) * OUT_C], (128, OUT_C))
            )
        )

    xh_packs, xl_packs = [], []
    for ti in range(TOK_SHARDS):
        x_sh = x[ti * TOK_C : (ti + 1) * TOK_C]
        if split:
            xh = x_sh.astype(ml_dtypes.bfloat16)
            xh_packs.append(_pack_x(xh))
            xl = (x_sh - xh.astype(np.float32)).astype(ml_dtypes.bfloat16)
            xl_packs.append(_pack_x(xl))
        else:
            xh_packs.append(_pack_x(x_sh))

    in_maps = []
    for c in range(N_CORES):
        ti, oi = divmod(c, OUT_SHARDS)
        m = {"xh": xh_packs[ti], "wt": wt_packs[oi], "br": bias_packs[oi]}
        if split:
            m["xl"] = xl_packs[ti]
        in_maps.append(m)
    return in_maps


def run(in_maps, trace=False, **kwargs):
    global _cached_nc
    from concourse.bass_utils import run_bass_kernel_spmd

    if _cached_nc is None:
        _cached_nc = build_nc()
    return run_bass_kernel_spmd(
        _cached_nc, in_maps, list(range(N_CORES)), trace=trace, **kwargs
    )


def gather(results):
    out = np.empty((N_TOK, D_OUT), dtype=np.float32)
    for c in range(N_CORES):
        ti, oi = divmod(c, OUT_SHARDS)
        out[ti * TOK_C : (ti + 1) * TOK_C, oi * OUT_C : (oi + 1) * OUT_C] = results[c][
            "out"
        ]
    return out


def kernel(x, weight, bias):
    res = run(prepare_in_maps(x, weight, bias), trace=False)
    return gather(res.results)


# revision 24
# speedup vs baseline: 1.8408x; 1.0421x over previous
"""Trainium2 kernel for BinaryLinear: out = x @ sign(clip(weight,-1,1)).T + bias.

Full shapes: x [8192, 4096] f32, weight [4096, 4096] f32, bias [4096] f32,
out [8192, 4096] f32.

Strategy (8 NeuronCores, no collectives needed):
  - Grid-shard tokens x out_features across the 8 cores; each core computes
    a disjoint output tile, host slices inputs / stitches outputs.
  - Binarized weights are exactly +-1 (bf16/f32r-exact). The matmul runs
    on the PE at 1 cycle/row using float32r operands (f32 bits, reduced-
    precision multiplier, ~2^-13 per-term error -> ~1e-4 rel overall).
  - Host packs x transposed+tiled so the contraction dim (in_features)
    lands on SBUF partitions with every DMA contiguous at line rate.
  - Per core: resident binarized-transposed weight slice in SBUF,
    stream 128-token blocks of xT, accumulate over K=4096 in PSUM,
    add bias on DVE while copying PSUM->SBUF, DMA out.

MODE:
  "f32r"  : single pass, f32r x f32r, 2x4 grid (tok x outf). ~1e-4 rel.
  "bf16x2": x split hi/lo into two bf16 passes, 4x2 grid. ~2e-6 rel,
            about 1.8x slower.
"""

import sys

if "/opt/trn_rl_repo" not in sys.path:
    sys.path.insert(0, "/opt/trn_rl_repo")

import ml_dtypes
import numpy as np

MODE = "f32r"

N_TOK, D_IN, D_OUT = 8192, 4096, 4096
if MODE == "f32r":
    TOK_SHARDS, OUT_SHARDS = 2, 4
else:
    TOK_SHARDS, OUT_SHARDS = 4, 2
N_CORES = TOK_SHARDS * OUT_SHARDS
TOK_C = N_TOK // TOK_SHARDS
OUT_C = D_OUT // OUT_SHARDS
MB = TOK_C // 128  # token blocks per core
KB = D_IN // 128  # contraction blocks
NF = 512  # matmul moving free dim (one fp32 PSUM bank)
NB = OUT_C // NF  # PSUM banks per token block

_cached_nc = None


def build_nc():
    import concourse.bacc as bacc
    import concourse.mybir as mybir
    import concourse.tile as tile

    dt = mybir.dt
    split = MODE == "bf16x2"
    mdt = dt.bfloat16 if split else dt.float32r

    nc = bacc.Bacc()
    xh_d = nc.dram_tensor("xh", [MB, 128, D_IN], mdt, kind="ExternalInput")
    if split:
        xl_d = nc.dram_tensor("xl", [MB, 128, D_IN], mdt, kind="ExternalInput")
    # weights always ship as bf16 (+-1 is exact); the f32r path upconverts
    # on-chip (DVE) so the weight prefetch moves half the bytes.
    wt_d = nc.dram_tensor("wt", [KB, 128, OUT_C], dt.bfloat16, kind="ExternalInput")
    br_d = nc.dram_tensor("br", [128, OUT_C], dt.float32, kind="ExternalInput")
    out_d = nc.dram_tensor("out", [TOK_C, OUT_C], dt.float32, kind="ExternalOutput")

    # First TRICKLE token-blocks are loaded before the weight stream and
    # their matmuls interleaved per k-block, so the PE computes while
    # weights arrive instead of idling at kernel start.
    TRICKLE = 0 if split else 3

    with tile.TileContext(nc) as tc:
        with (
            tc.tile_pool(name="wts", bufs=1) as wpool,
            tc.tile_pool(name="wstage", bufs=2) as spool,
            tc.tile_pool(name="bias", bufs=1) as bpool,
            tc.tile_pool(name="xin", bufs=max(2, TRICKLE + 1)) as xpool,
            tc.tile_pool(name="outp", bufs=1 if not split else 2) as opool,
            tc.tile_pool(name="psum", bufs=8, space="PSUM") as ppool,
        ):

            def load_x(m):
                xh_m = xpool.tile([128, D_IN], mdt, name=f"xh_{m}", tag="xh")
                nc.sync.dma_start(xh_m[:], xh_d[m])
                passes = [xh_m]
                if split:
                    xl_m = xpool.tile([128, D_IN], mdt, name=f"xl_{m}", tag="xl")
                    nc.sync.dma_start(xl_m[:], xl_d[m])
                    passes.append(xl_m)
                return passes

            def alloc_ps(m):
                return [
                    ppool.tile([128, NF], dt.float32, name=f"ps_{m}_{n}", tag="ps")
                    for n in range(NB)
                ]

            def emit_mms(kb, passes, ps):
                n_half = len(passes)
                for hi, xm in enumerate(passes):
                    lhs = xm[:, kb * 128 : (kb + 1) * 128]
                    for n in range(NB):
                        rhs = wts[kb][:, n * NF : (n + 1) * NF]
                        nc.tensor.matmul(
                            ps[n][:],
                            lhs,
                            rhs,
                            start=(kb == 0 and hi == 0),
                            stop=(kb == KB - 1 and hi == n_half - 1),
                        )

            def flush(m, ps):
                out_t = opool.tile([128, OUT_C], dt.float32, name=f"o_{m}", tag="out")
                for n in range(NB):
                    nc.vector.tensor_tensor(
                        out_t[:, n * NF : (n + 1) * NF],
                        ps[n][:],
                        bias_s[:, n * NF : (n + 1) * NF],
                        mybir.AluOpType.add,
                    )
                nc.sync.dma_start(out_d[m * 128 : (m + 1) * 128, :], out_t[:])

            def load_w(kb):
                if split:
                    w = wpool.tile([128, OUT_C], mdt, name=f"wt{kb}", tag=f"wt{kb}")
                    nc.sync.dma_start(w[:], wt_d[kb])
                else:
                    # bf16 DMA + DVE upconvert; matmul bitcasts to f32r
                    stage = spool.tile(
                        [128, OUT_C], dt.bfloat16, name=f"ws{kb}", tag="wstage"
                    )
                    nc.sync.dma_start(stage[:], wt_d[kb])
                    w = wpool.tile(
                        [128, OUT_C], dt.float32r, name=f"wt{kb}", tag=f"wt{kb}"
                    )
                    nc.vector.tensor_copy(w[:], stage[:])
                wts.append(w)

            # Interleave trickle-x loads with the weight stream so both the
            # PE's first operands and the early k-blocks arrive ASAP.
            wts = []
            trickle_x = {}
            if TRICKLE:
                trickle_x[0] = load_x(0)
                for kb in range(0, 6):
                    load_w(kb)
                trickle_x[1] = load_x(1)
                for kb in range(6, 14):
                    load_w(kb)
                trickle_x[2] = load_x(2)
                for kb in range(14, KB):
                    load_w(kb)
            else:
                for kb in range(KB):
                    load_w(kb)
            bias_s = bpool.tile([128, OUT_C], dt.float32, name="bias_s")
            nc.sync.dma_start(bias_s[:], br_d[:])
            for m in range(3, TRICKLE):
                trickle_x[m] = load_x(m)

            if TRICKLE:
                trickle_ps = {m: alloc_ps(m) for m in range(TRICKLE)}
                # m-major kb-chunks ordered to match DMA arrivals of
                # (xt_m, wt[kb]) so the PE never waits on a late tile.
                sched = [
                    (0, 0, 6),
                    (1, 0, 6),
                    (0, 6, 14),
                    (1, 6, 14),
                    (2, 0, 14),
                    (0, 14, KB),
                    (1, 14, KB),
                    (2, 14, KB),
                ] + [(m, 0, KB) for m in range(3, TRICKLE)]
                for m, k0, k1 in sched:
                    for kb in range(k0, k1):
                        emit_mms(kb, trickle_x[m], trickle_ps[m])
                for m in range(TRICKLE):
                    flush(m, trickle_ps[m])

            for m in range(TRICKLE, MB):
                passes = load_x(m)
                ps = alloc_ps(m)
                for kb in range(KB):
                    emit_mms(kb, passes, ps)
                flush(m, ps)

    nc.compile()
    return nc


def _pack_x(a):
    """[TOK_C, D_IN] -> [MB, 128, D_IN] with layout [m, p, (kb t)]:
    packed[m, p, kb*128 + t] = a[m*128 + t, kb*128 + p]."""
    return np.ascontiguousarray(
        a.reshape(MB, 128, KB, 128).transpose(0, 3, 2, 1)
    ).reshape(MB, 128, D_IN)


def prepare_in_maps(x, weight, bias):
    x = np.asarray(x, dtype=np.float32)
    weight = np.asarray(weight, dtype=np.float32)
    bias = np.asarray(bias, dtype=np.float32)
    split = MODE == "bf16x2"
    npdt = ml_dtypes.bfloat16 if split else np.float32

    bw = np.where(weight >= 0, np.float32(1.0), np.float32(-1.0))

    wt_packs, bias_packs = [], []
    for oi in range(OUT_SHARDS):
        w_sh = bw[oi * OUT_C : (oi + 1) * OUT_C]  # [OUT_C, D_IN]
        wt = np.ascontiguousarray(w_sh.T).astype(ml_dtypes.bfloat16)
        wt_packs.append(wt.reshape(KB, 128, OUT_C))
        bias_packs.append(
            np.ascontiguousarray(
                np.broadcast_to(bias[oi * OUT_C : (oi + 1) * OUT_C], (128, OUT_C))
            )
        )

    xh_packs, xl_packs = [], []
    for ti in range(TOK_SHARDS):
        x_sh = x[ti * TOK_C : (ti + 1) * TOK_C]
        if split:
            xh = x_sh.astype(ml_dtypes.bfloat16)
            xh_packs.append(_pack_x(xh))
            xl = (x_sh - xh.astype(np.float32)).astype(ml_dtypes.bfloat16)
            xl_packs.append(_pack_x(xl))
        else:
            xh_packs.append(_pack_x(x_sh))

    in_maps = []
    for c in range(N_CORES):
        ti, oi = divmod(c, OUT_SHARDS)
        m = {"xh": xh_packs[ti], "wt": wt_packs[oi], "br": bias_packs[oi]}
        if split:
            m["xl"] = xl_packs[ti]
        in_maps.append(m)
    return in_maps


def run(in_maps, trace=False, **kwargs):
    global _cached_nc
    from concourse.bass_utils import run_bass_kernel_spmd

    if _cached_nc is None:
        _cached_nc = build_nc()
    return run_bass_kernel_spmd(
        _cached_nc, in_maps, list(range(N_CORES)), trace=trace, **kwargs
    )


def gather(results):
    out = np.empty((N_TOK, D_OUT), dtype=np.float32)
    for c in range(N_CORES):
        ti, oi = divmod(c, OUT_SHARDS)
        out[ti * TOK_C : (ti + 1) * TOK_C, oi * OUT_C : (oi + 1) * OUT_C] = results[c][
            "out"
        ]
    return out


def kernel(x, weight, bias):
    res = run(prepare_in_maps(x, weight, bias), trace=False)
    return gather(res.results)
